# revision 9
# baseline (speedup 1.0000x reference)
"""Batched Viterbi decode (CRF inference) on 8 Trainium2 NeuronCores — v3.

Data-parallel over batch (64 seqs/core) with a PAIR-SPLIT layout: each
sequence occupies two adjacent SBUF partitions (2s owns next-tag pages
j=0..15, 2s+1 owns j=16..31), so every per-step O(L^2) DVE op runs on
[128, 512] instead of [64, 1024] — half the per-partition free size.
stream_shuffle (partition permute within 32-lane quadrants; pairs never
cross a quadrant) rebuilds the replicated 32-wide alpha each step and
replicates backpointers into a full-width ring on both partitions.

Forward step t (j-page-local, i = global prev-tag 0..31):
    sc[p,(jj,i)] = alpha_full[p,i] + transT_own[p,(jj,i)]   TT add (bcast AP)
    R = per-page running max of sc                          TTS scan (rstv reset)
    M_own[p,jj] = R[p,(jj,31)]; cand = M_own + e_own        TT add (strided)
    alpha_own = where(m_t, cand, alpha_own)                 copy_predicated
    alpha_full <- 2x stream_shuffle(alpha_own)
    ltt = [R < M_own bcast] (fp16); bp_own = sum_i ltt (u8) = first-argmax
    ring[t] <- 2x stream_shuffle(bp_own)                    (u8, both halves)

The per-step instruction stream is software-pipelined: the bp-side ops of
step t-1/t-2 are interleaved into step t's serial alpha chain so that every
same-engine RAW semaphore wait has >=195ns of independent work between
producer and consumer (sem update propagation window) and costs ~nothing.
lt and reduce are emitted as two halves each purely to create filler.

Backtrace: tag_t = ring[t+1][tag_{t+1}] via scalar_tensor_tensor gather
(iota==tag)*ring summed; ring read directly as u8 (mixed-dtype STT).

All f32 adds/compares run in the same order on the same values as the jax
reference -> bit-exact paths incl. first-argmax tie-breaking.

v3 wall-clock changes (the metric is the warm end-to-end run, which is
dominated by host->device transfer over the tunnel and per-call jit
compile, not device execution):
  - persistent jax compilation cache -> the per-call XLA+neuronx recompile
    (~2-4s) becomes a disk cache hit.
  - emissions past each sequence's length are zeroed host-side; the wire
    compresses those runs (measured rand 53MB/s vs zeros 90MB/s), cutting
    the 134MB x transfer time ~25%.
  - masks and tables are built on device from seq_lengths [P,1] + an 8KB
    transition table (broadcast-DMA) instead of shipping ~1.3MB/core of
    precomputed masks/tables.
  - paths output is u8 (tags 0..31) instead of i32: 4x smaller fetch.
"""

import os
import sys

for p in ("/opt/trn_rl_repo", "/opt/pypackages"):
    if p not in sys.path:
        sys.path.insert(0, p)

from contextlib import ExitStack

import numpy as np

import jax

_CACHE_DIR = "/root/.cache/jax_bass_cache"
try:
    os.makedirs(_CACHE_DIR, exist_ok=True)
    jax.config.update("jax_compilation_cache_dir", _CACHE_DIR)
    jax.config.update("jax_persistent_cache_min_compile_time_secs", 0.0)
    jax.config.update("jax_persistent_cache_min_entry_size_bytes", 0)
except Exception:
    pass

import concourse.bass as bass
from concourse import mybir
from concourse.bass_utils import run_bass_kernel_spmd

A = mybir.AluOpType
DT = mybir.dt
AX = mybir.AxisListType

B, T, L = 512, 2048, 32
NCORES = 8
SEQ = B // NCORES          # 64 sequences per core
P = 2 * SEQ                # 128 partitions, 2 per sequence
H = L // 2                 # 16 own pages per partition
NEG = -1.0e30
CHUNK = 64                 # forward steps per emission DMA chunk
CE = CHUNK * H             # chunk elems per partition

EVEN = [q * 2 for q in range(16) for _ in (0, 1)]     # pair -> even member
ODD = [q * 2 + 1 for q in range(16) for _ in (0, 1)]  # pair -> odd member


_NC_CACHE = {}


def build_program(T_=T):
    if T_ in _NC_CACHE:
        return _NC_CACHE[T_]
    assert T_ % CHUNK == 0
    nch = T_ // CHUNK

    nc = bass.Bass()
    nc.detect_race_conditions = False
    x = nc.declare_dram_parameter("x", [P, T_ * H], DT.float32, isOutput=False)
    trep = nc.declare_dram_parameter("trep", [P, H * L], DT.float32, isOutput=False)
    lens = nc.declare_dram_parameter("lens", [P, 1], DT.float32, isOutput=False)
    pout = nc.declare_dram_parameter("paths", [P, T_], DT.uint8, isOutput=True)
    with ExitStack() as ctx:
        e = ctx.enter_context
        trep_sb = e(nc.sbuf_tensor([P, H * L], DT.float32))
        rstv_sb = e(nc.sbuf_tensor([P, H * L], DT.float32))
        iotf_sb = e(nc.sbuf_tensor([P, L], DT.float32))
        iotu_sb = e(nc.sbuf_tensor([P, L], DT.uint8))
        lens_sb = e(nc.sbuf_tensor([P, 1], DT.float32))
        it_sb = e(nc.sbuf_tensor([P, T_], DT.float32))
        nsm_sb = e(nc.sbuf_tensor([P, T_], DT.uint8))
        m_sb = e(nc.sbuf_tensor([P, T_], DT.uint8))
        xt_a = e(nc.sbuf_tensor([P, CE], DT.float32))
        xt_b = e(nc.sbuf_tensor([P, CE], DT.float32))
        aown = e(nc.sbuf_tensor([P, H], DT.float32))
        afull = e(nc.sbuf_tensor([P, L], DT.float32))
        sc = e(nc.sbuf_tensor([P, H * L], DT.float32))
        R = e(nc.sbuf_tensor([P, H * L], DT.float32))
        ltt = e(nc.sbuf_tensor([P, 2 * H * L], DT.float16))   # double-buffered
        tt2f = e(nc.sbuf_tensor([P, 2 * H * H], DT.float16))  # tree level, dbl
        bp_dual = e(nc.sbuf_tensor([P, 4 * H], DT.uint8))     # 4-slot rotation
        cand = e(nc.sbuf_tensor([P, H], DT.float32))
        ring = e(nc.sbuf_tensor([P, T_ * L], DT.uint8))
        paths = e(nc.sbuf_tensor([P, T_], DT.float32))
        outi = e(nc.sbuf_tensor([P, T_], DT.uint8))
        lt32 = e(nc.sbuf_tensor([P, L], DT.float32))
        junk = e(nc.sbuf_tensor([P, L], DT.float32))
        tbl_sem = e(nc.semaphore("tbl_sem"))
        xa_sem = e(nc.semaphore("xa_sem"))
        xb_sem = e(nc.semaphore("xb_sem"))
        out_sem = e(nc.semaphore("out_sem"))
        dve_sem = e(nc.semaphore("dve_sem"))

        xt_ab = [xt_a, xt_b]
        trep3 = trep_sb[:].rearrange("p (j i) -> p j i", i=L)
        sc3 = sc[:].rearrange("p (j i) -> p j i", i=L)
        R3 = R[:].rearrange("p (j i) -> p j i", i=L)
        rstv3 = rstv_sb[:].rearrange("p (j i) -> p j i", i=L)
        lt4 = ltt[:].rearrange("p (b j i) -> p b j i", b=2, i=L)   # [P,2,H,L]
        tt4 = tt2f[:].rearrange("p (b j k) -> p b j k", b=2, k=H)  # [P,2,H,H]
        bp4 = bp_dual[:].rearrange("p (s h) -> p s h", h=H)        # [P,4,H]
        Mv = R3[:, :, L - 1 : L]
        afull_b = afull[:].unsqueeze(1).broadcast_to([P, H, L])
        # pair-major ring: [pair q, half h, slot-in-pair w, k]; slot t>=1
        # lives at (q, w) = ((t-1)//2, (t-1)%2). Keeps every AP stride <= 64
        # (16-bit ISA stride fields) while dual-slot shuffle targets stay
        # flat contiguous 32-byte runs.
        ring4 = ring[:].rearrange("p (q h w k) -> p q h w k", h=2, w=2, k=H)
        iotf2 = iotf_sb[:].rearrange("p (h k) -> p h k", k=H)
        junk2 = junk[:].rearrange("p (h k) -> p h k", k=H)
        lens_bc = lens_sb[:, 0:1].broadcast_to([P, T_])
        HH = H // 2  # half of the own pages, for lt/reduce splitting
        N_SETUP = 7  # gpsimd setup ops counted on tbl_sem

        with nc.Block() as block:
            marks = {}
            total = [0]
            slice_marks = []

            @block.vector
            def _(v):
                n = [0]

                def S(inst):
                    inst.then_inc(dve_sem, 1)
                    n[0] += 1
                    return n[0]

                def W(k):
                    if k is not None:
                        v.wait_ge(dve_sem, k)

                v.wait_ge(tbl_sem, 16 * N_SETUP)
                # masks from lens: m[t] = t < len (used at t>=0),
                # nsm[t] = t >= len (only read at t>=1, where it equals the
                # reference's "masked step" indicator exactly)
                with nc.allow_low_precision(reason="0/1 mask in u8"):
                    S(v.tensor_tensor(out=m_sb[:], in0=it_sb[:], in1=lens_bc, op=A.is_lt))
                    S(v.tensor_tensor(out=nsm_sb[:], in0=it_sb[:], in1=lens_bc, op=A.is_ge))
                v.wait_ge(xa_sem, 16)  # chunk 0
                i_aown = S(v.tensor_copy(aown[:], xt_a[:, 0:H]))
                W(i_aown)
                S(v.stream_shuffle(afull[:, 0:H], aown[:], EVEN))
                i_shufO = S(v.stream_shuffle(afull[:, H:L], aown[:], ODD))

                # software-pipelined forward loop.
                # iteration t emits: alpha-chain(t), lt halves + reduce halves
                # of (t-1), ring shuffles of (t-2). Producer indices tracked
                # for exact wait targets.
                idx_pred = {}
                idx_shufO = {1: i_shufO}
                idx_scan = {}
                idx_ltb = {}
                idx_tt = {}
                idx_redb = {}

                def bpcol(s):
                    # bp_dual rotation: slot s lives at column (s+1)%4 so
                    # (odd, even) step pairs occupy contiguous column pairs.
                    return (s + 1) % 4

                for t in range(1, T_ + 3):
                    cur = t <= T_ - 1
                    c = t // CHUNK if cur else 0
                    u = t % CHUNK
                    xt = xt_ab[c % 2]
                    if cur and u == 0:
                        # first step of chunk c: ensure its DMA landed
                        # (issued ~CHUNK steps ago -> free wait)
                        if c % 2 == 0:
                            v.wait_ge(xa_sem, 16 * (c // 2 + 1))
                        else:
                            v.wait_ge(xb_sem, 16 * ((c - 1) // 2 + 1))

                    # --- alpha chain of t, software-pipelined with the
                    # backpointer side of steps t-1 / t-2 / t-3 / t-4 ---
                    if cur and t > 1:
                        W(idx_pred[t - 1])
                        S(v.stream_shuffle(afull[:, 0:H], aown[:], EVEN))
                        idx_shufO[t] = S(v.stream_shuffle(afull[:, H:L], aown[:], ODD))
                    # filler: lt first half of t-1 (reads R before scan(t))
                    s1 = t - 1
                    if 1 <= s1 <= T_ - 1:
                        W(idx_scan[s1])
                        S(v.tensor_tensor(
                            out=lt4[:, s1 % 2, 0:HH, :], in0=R3[:, 0:HH, :],
                            in1=Mv[:, 0:HH, :].broadcast_to([P, HH, L]), op=A.is_lt))
                    if cur:
                        W(idx_shufO[t])
                        i_add = S(v.tensor_tensor(out=sc3, in0=afull_b, in1=trep3, op=A.add))
                    # filler: lt second half of t-1
                    if 1 <= s1 <= T_ - 1:
                        idx_ltb[s1] = S(v.tensor_tensor(
                            out=lt4[:, s1 % 2, HH:H, :], in0=R3[:, HH:H, :],
                            in1=Mv[:, HH:H, :].broadcast_to([P, HH, L]), op=A.is_lt))
                    if cur:
                        W(i_add)
                        idx_scan[t] = S(v.tensor_tensor_scan(
                            out=R[:], data0=rstv_sb[:], data1=sc[:],
                            initial=0.0, op0=A.add, op1=A.max))
                    # filler: fp16 tree level of t-2
                    s2 = t - 2
                    if 1 <= s2 <= T_ - 1:
                        W(idx_ltb[s2])
                        idx_tt[s2] = S(v.tensor_tensor(
                            out=tt4[:, s2 % 2], in0=lt4[:, s2 % 2, :, 0:H],
                            in1=lt4[:, s2 % 2, :, H:L], op=A.add))
                    if cur:
                        W(idx_scan[t])
                        i_cand = S(v.tensor_tensor(
                            out=cand[:].unsqueeze(2), in0=Mv,
                            in1=xt[:, u * H : (u + 1) * H].unsqueeze(2), op=A.add))
                    # filler: reduce first half of t-3
                    s3 = t - 3
                    if 1 <= s3 <= T_ - 1:
                        W(idx_tt[s3])
                        with nc.allow_low_precision(reason="bp count <= 32, exact in u8"):
                            S(v.tensor_reduce(
                                out=bp4[:, bpcol(s3), 0:HH],
                                in_=tt4[:, s3 % 2, 0:HH, :], axis=AX.X, op=A.add))
                    if cur:
                        W(i_cand)
                        inst = v.copy_predicated(
                            out=aown[:],
                            mask=m_sb[:, t : t + 1].broadcast_to([P, H]),
                            data=cand[:])
                        idx_pred[t] = S(inst)
                        if u == CHUNK - 1:
                            # chunk c fully consumed by DVE at this point
                            marks[c] = n[0]
                    # tail filler: reduce second half of t-3
                    if 1 <= s3 <= T_ - 1:
                        with nc.allow_low_precision(reason="bp count <= 32, exact in u8"):
                            idx_redb[s3] = S(v.tensor_reduce(
                                out=bp4[:, bpcol(s3), HH:H],
                                in_=tt4[:, s3 % 2, HH:H, :], axis=AX.X, op=A.add))
                    # tail: dual-slot ring shuffles for the (s4-1, s4) pair
                    s4 = t - 4
                    if s4 >= 2 and s4 % 2 == 0 and s4 <= T_ - 1:
                        W(idx_redb[s4])
                        base = s4 % 4
                        bpp = bp_dual[:, base * H : (base + 2) * H]
                        q = s4 // 2 - 1
                        S(v.stream_shuffle(ring[:, q * 4 * H : q * 4 * H + 2 * H], bpp, EVEN))
                        S(v.stream_shuffle(ring[:, q * 4 * H + 2 * H : (q + 1) * 4 * H], bpp, ODD))

                # dangling odd final slot T-1 (2047): single-slot ring shuffles
                W(idx_redb[T_ - 1])
                bpl = bp4[:, bpcol(T_ - 1), :]
                ql = (T_ - 2) // 2
                S(v.stream_shuffle(ring4[:, ql, 0, 0, :], bpl, EVEN))
                S(v.stream_shuffle(ring4[:, ql, 1, 0, :], bpl, ODD))

                # --- identity backpointers on masked steps ---
                W(n[0])
                nq = T_ // 2
                for l_ in range(L):
                    for w in range(2):
                        qcnt = nq if w == 0 else nq - 1  # slot 2q+1+w <= T-1
                        S(v.copy_predicated(
                            out=ring4[:, 0:qcnt, l_ // H, w, l_ % H],
                            mask=nsm_sb[:, 1 + w : 2 + w + 2 * (qcnt - 1) : 2],
                            data=iotu_sb[:, l_ : l_ + 1].broadcast_to([P, qcnt])))

                # --- final argmax: paths[:, T-1] (first argmax of afull) ---
                # rebuild afull: the loop's last shuffles ran before pred(T-1),
                # so len==T sequences have a stale second... both halves.
                W(idx_pred[T_ - 1])
                S(v.stream_shuffle(afull[:, 0:H], aown[:], EVEN))
                S(v.stream_shuffle(afull[:, H:L], aown[:], ODD))
                W(n[0])
                S(v.tensor_tensor_scan(
                    out=lt32[:], data0=rstv_sb[:, 0:L], data1=afull[:],
                    initial=0.0, op0=A.add, op1=A.max))
                W(n[0])
                S(v.tensor_tensor(
                    out=junk[:], in0=lt32[:],
                    in1=lt32[:, L - 1 : L].broadcast_to([P, L]), op=A.is_lt))
                W(n[0])
                S(v.tensor_reduce(
                    out=paths[:, T_ - 1 : T_], in_=junk[:], axis=AX.X, op=A.add))

                # --- backtrace, with incremental masked-output slices ---
                # once the chase passes t, paths[t..] is final: fuse
                # mask*convert into one mixed TT (f32*u8 -> u8, exact for
                # tag ints) per slice and hand it to the DMA early so the
                # output transfer hides under the remaining chase.
                SLICE = max(64, T_ // 8)
                for t in range(T_ - 2, -1, -1):
                    W(n[0])
                    S(v.scalar_tensor_tensor(
                        out=junk2,
                        in0=iotf2,
                        scalar=paths[:, t + 1 : t + 2],
                        in1=ring4[:, t // 2, :, t % 2, :],
                        op0=A.is_equal,
                        op1=A.mult,
                        accum_out=paths[:, t : t + 1]))
                    if t % SLICE == 0:
                        W(n[0])
                        hi = T_ if t + SLICE > T_ - SLICE else t + SLICE
                        with nc.allow_low_precision(reason="tags 0..31 exact in u8"):
                            k = S(v.tensor_tensor(
                                out=outi[:, t:hi], in0=paths[:, t:hi],
                                in1=m_sb[:, t:hi], op=A.mult))
                        slice_marks.append((t, hi, k))
                total[0] = n[0]
                v.wait_ge(out_sem, 16 * len(slice_marks))

            @block.gpsimd
            def _(g):
                g.dma_start(trep_sb[:], trep[:]).then_inc(tbl_sem, 16)
                g.dma_start(lens_sb[:], lens[:]).then_inc(tbl_sem, 16)
                g.iota(it_sb[:], [[1, T_]], channel_multiplier=0,
                       allow_small_or_imprecise_dtypes=True).then_inc(tbl_sem, 16)
                g.iota(iotf_sb[:], [[1, L]], channel_multiplier=0,
                       allow_small_or_imprecise_dtypes=True).then_inc(tbl_sem, 16)
                with nc.allow_low_precision(reason="iota 0..31 exact in u8"):
                    g.tensor_copy(iotu_sb[:], iotf_sb[:]).then_inc(tbl_sem, 16)
                g.memset(rstv_sb[:], 0.0).then_inc(tbl_sem, 16)
                g.memset(rstv3[:, :, 0:1], NEG).then_inc(tbl_sem, 16)
                for c in range(nch):
                    if c >= 2:
                        g.wait_ge(dve_sem, marks[c - 2])
                    g.dma_start(
                        xt_ab[c % 2][:], x[:, c * CE : (c + 1) * CE]
                    ).then_inc(xa_sem if c % 2 == 0 else xb_sem, 16)
                for (t0, hi, k) in slice_marks:
                    g.wait_ge(dve_sem, k)
                    g.dma_start(pout[:, t0:hi], outi[:, t0:hi]).then_inc(out_sem, 16)

    # the program is immutable from here on; memoize its BIR serialization
    # (the jit lowering re-serializes nc on every call otherwise, ~0.25s)
    blob = nc.to_json_bytes()
    nc.to_json_bytes = lambda: blob
    _NC_CACHE[T_] = nc
    return nc


def prepare_in_maps(np_inputs, T_=T):
    inputs = np.asarray(np_inputs["inputs"], dtype=np.float32)
    seq_lengths = np.asarray(np_inputs["seq_lengths"], dtype=np.int32)
    trans_params = np.asarray(np_inputs["trans_params"], dtype=np.float32)

    tT = np.ascontiguousarray(trans_params.T)  # [j, i]
    trep = np.zeros((P, H, L), np.float32)
    trep[0::2] = tT[None, 0:H, :]
    trep[1::2] = tT[None, H:L, :]
    trep = trep.reshape(P, H * L)

    t_idx = np.arange(T_, dtype=np.int64)
    in_maps = []
    for k in range(NCORES):
        xs = inputs[k * SEQ : (k + 1) * SEQ]          # [SEQ, T, L]
        ls = seq_lengths[k * SEQ : (k + 1) * SEQ]     # [SEQ]
        # pair-split emissions: partition 2s = j 0..15, 2s+1 = j 16..31
        xo = np.empty((P, T_, H), np.float32)
        xo[0::2] = xs[:, :, 0:H]
        xo[1::2] = xs[:, :, H:L]
        # zero the padded region (never read by the decode; long zero runs
        # compress on the wire -> measurably faster host->device transfer)
        m = (t_idx[None, :] < ls[:, None])            # [SEQ, T]
        xo[np.repeat(~m, 2, axis=0)] = 0
        lens = np.repeat(ls, 2).astype(np.float32).reshape(P, 1)
        in_maps.append({
            "x": xo.reshape(P, T_ * H),
            "trep": trep,
            "lens": lens,
        })
    return in_maps, None


def assemble_output(results):
    paths = np.stack(
        [results[k]["paths"][0::2, :] for k in range(NCORES)], axis=0
    )
    return paths.reshape(B, T).astype(np.int32)


def kernel(inputs, seq_lengths, trans_params):
    nc = build_program()
    in_maps, _ = prepare_in_maps(
        {
            "inputs": inputs,
            "seq_lengths": seq_lengths,
            "trans_params": trans_params,
        }
    )
    res = run_bass_kernel_spmd(nc, in_maps, list(range(NCORES)))
    return assemble_output(res.results)


# revision 13
# speedup vs baseline: 1.8618x; 1.8618x over previous
"""Batched Viterbi decode (CRF inference) on 8 Trainium2 NeuronCores — v3.

Data-parallel over batch (64 seqs/core) with a PAIR-SPLIT layout: each
sequence occupies two adjacent SBUF partitions (2s owns next-tag pages
j=0..15, 2s+1 owns j=16..31), so every per-step O(L^2) DVE op runs on
[128, 512] instead of [64, 1024] — half the per-partition free size.
stream_shuffle (partition permute within 32-lane quadrants; pairs never
cross a quadrant) rebuilds the replicated 32-wide alpha each step and
replicates backpointers into a full-width ring on both partitions.

Forward step t (j-page-local, i = global prev-tag 0..31):
    sc[p,(jj,i)] = alpha_full[p,i] + transT_own[p,(jj,i)]   TT add (bcast AP)
    R = per-page running max of sc                          TTS scan (rstv reset)
    M_own[p,jj] = R[p,(jj,31)]; cand = M_own + e_own        TT add (strided)
    alpha_own = where(m_t, cand, alpha_own)                 copy_predicated
    alpha_full <- 2x stream_shuffle(alpha_own)
    ltt = [R < M_own bcast] (fp16); bp_own = sum_i ltt (u8) = first-argmax
    ring[t] <- 2x stream_shuffle(bp_own)                    (u8, both halves)

The per-step instruction stream is software-pipelined: the bp-side ops of
step t-1/t-2 are interleaved into step t's serial alpha chain so that every
same-engine RAW semaphore wait has >=195ns of independent work between
producer and consumer (sem update propagation window) and costs ~nothing.
lt and reduce are emitted as two halves each purely to create filler.

Backtrace: tag_t = ring[t+1][tag_{t+1}] via scalar_tensor_tensor gather
(iota==tag)*ring summed; ring read directly as u8 (mixed-dtype STT).

All f32 adds/compares run in the same order on the same values as the jax
reference -> bit-exact paths incl. first-argmax tie-breaking.

v3 wall-clock changes (the metric is the warm end-to-end run, which is
dominated by host->device transfer over the tunnel and per-call jit
compile, not device execution):
  - persistent jax compilation cache -> the per-call XLA+neuronx recompile
    (~2-4s) becomes a disk cache hit.
  - emissions past each sequence's length are zeroed host-side; the wire
    compresses those runs (measured rand 53MB/s vs zeros 90MB/s), cutting
    the 134MB x transfer time ~25%.
  - masks and tables are built on device from seq_lengths [P,1] + an 8KB
    transition table (broadcast-DMA) instead of shipping ~1.3MB/core of
    precomputed masks/tables.
  - paths output is u8 (tags 0..31) instead of i32: 4x smaller fetch.
"""

import os
import sys

for p in ("/opt/trn_rl_repo", "/opt/pypackages"):
    if p not in sys.path:
        sys.path.insert(0, p)

from contextlib import ExitStack

import numpy as np

import jax

_CACHE_DIR = "/root/.cache/jax_bass_cache"
try:
    os.makedirs(_CACHE_DIR, exist_ok=True)
    jax.config.update("jax_compilation_cache_dir", _CACHE_DIR)
    jax.config.update("jax_persistent_cache_min_compile_time_secs", 0.0)
    jax.config.update("jax_persistent_cache_min_entry_size_bytes", 0)
except Exception:
    pass

import concourse.bass as bass
from concourse import mybir
from concourse.bass_utils import run_bass_kernel_spmd

A = mybir.AluOpType
DT = mybir.dt
AX = mybir.AxisListType

B, T, L = 512, 2048, 32
NCORES = 8
SEQ = B // NCORES          # 64 sequences per core
P = 2 * SEQ                # 128 partitions, 2 per sequence
H = L // 2                 # 16 own pages per partition
NEG = -1.0e30
CHUNK = 64                 # forward steps per emission DMA chunk
CE = CHUNK * H             # chunk elems per partition

EVEN = [q * 2 for q in range(16) for _ in (0, 1)]     # pair -> even member
ODD = [q * 2 + 1 for q in range(16) for _ in (0, 1)]  # pair -> odd member


_NC_CACHE = {}


def chunk_rows(seq_lengths, T_=T):
    """Global length-sorted striping: rank r -> core r%8, pair slot r//8.

    Returns (order, pcs): `order[j*8+k]` = global seq index held by core k's
    pair j; `pcs[c]` = partitions (rows) of emission data chunk c carries —
    identical across cores so one SPMD program serves all 8. Rows past a
    core's own need are zero-filled and masked. Packing chunk c down to the
    sequences still alive at step 64c (sorted, so they occupy the leading
    partition pairs) halves the bytes shipped over the tunnel for uniform
    random lengths.
    """
    seq_lengths = np.asarray(seq_lengths)
    order = np.argsort(-seq_lengths, kind="stable")
    ranks2d = seq_lengths[order].reshape(SEQ, NCORES)  # [pair j, core k]
    pcs = []
    for c in range(T_ // CHUNK):
        need = (ranks2d > CHUNK * c).sum(axis=0)  # per core
        pcs.append(2 * max(int(need.max()), 1))
    return order, tuple(pcs)


def build_program(T_=T, pcs=None):
    if pcs is None:
        pcs = (P,) * (T_ // CHUNK)
    key = (T_, pcs)
    if key in _NC_CACHE:
        return _NC_CACHE[key]
    assert T_ % CHUNK == 0
    nch = T_ // CHUNK
    assert len(pcs) == nch and pcs[0] == P
    total_rows = sum(pcs)
    row_off = np.concatenate([[0], np.cumsum(pcs)]).astype(int)

    nc = bass.Bass()
    nc.detect_race_conditions = False
    x = nc.declare_dram_parameter("x", [total_rows, CE], DT.float32, isOutput=False)
    trep = nc.declare_dram_parameter("trep", [P, H * L], DT.float32, isOutput=False)
    lens = nc.declare_dram_parameter("lens", [P, 1], DT.float32, isOutput=False)
    pout = nc.declare_dram_parameter("paths", [P, T_], DT.uint8, isOutput=True)
    with ExitStack() as ctx:
        e = ctx.enter_context
        trep_sb = e(nc.sbuf_tensor([P, H * L], DT.float32))
        rstv_sb = e(nc.sbuf_tensor([P, H * L], DT.float32))
        iotf_sb = e(nc.sbuf_tensor([P, L], DT.float32))
        iotu_sb = e(nc.sbuf_tensor([P, L], DT.uint8))
        lens_sb = e(nc.sbuf_tensor([P, 1], DT.float32))
        it_sb = e(nc.sbuf_tensor([P, T_], DT.float32))
        nsm_sb = e(nc.sbuf_tensor([P, T_], DT.uint8))
        m_sb = e(nc.sbuf_tensor([P, T_], DT.uint8))
        xt_a = e(nc.sbuf_tensor([P, CE], DT.float32))
        xt_b = e(nc.sbuf_tensor([P, CE], DT.float32))
        aown = e(nc.sbuf_tensor([P, H], DT.float32))
        afull = e(nc.sbuf_tensor([P, L], DT.float32))
        sc = e(nc.sbuf_tensor([P, H * L], DT.float32))
        R = e(nc.sbuf_tensor([P, H * L], DT.float32))
        ltt = e(nc.sbuf_tensor([P, 2 * H * L], DT.float16))   # double-buffered
        tt2f = e(nc.sbuf_tensor([P, 2 * H * H], DT.float16))  # tree level, dbl
        bp_dual = e(nc.sbuf_tensor([P, 4 * H], DT.uint8))     # 4-slot rotation
        cand = e(nc.sbuf_tensor([P, H], DT.float32))
        ring = e(nc.sbuf_tensor([P, T_ * L], DT.uint8))
        paths = e(nc.sbuf_tensor([P, T_], DT.float32))
        outi = e(nc.sbuf_tensor([P, T_], DT.uint8))
        lt32 = e(nc.sbuf_tensor([P, L], DT.float32))
        junk = e(nc.sbuf_tensor([P, L], DT.float32))
        tbl_sem = e(nc.semaphore("tbl_sem"))
        xa_sem = e(nc.semaphore("xa_sem"))
        xb_sem = e(nc.semaphore("xb_sem"))
        out_sem = e(nc.semaphore("out_sem"))
        dve_sem = e(nc.semaphore("dve_sem"))

        xt_ab = [xt_a, xt_b]
        trep3 = trep_sb[:].rearrange("p (j i) -> p j i", i=L)
        sc3 = sc[:].rearrange("p (j i) -> p j i", i=L)
        R3 = R[:].rearrange("p (j i) -> p j i", i=L)
        rstv3 = rstv_sb[:].rearrange("p (j i) -> p j i", i=L)
        lt4 = ltt[:].rearrange("p (b j i) -> p b j i", b=2, i=L)   # [P,2,H,L]
        tt4 = tt2f[:].rearrange("p (b j k) -> p b j k", b=2, k=H)  # [P,2,H,H]
        bp4 = bp_dual[:].rearrange("p (s h) -> p s h", h=H)        # [P,4,H]
        Mv = R3[:, :, L - 1 : L]
        afull_b = afull[:].unsqueeze(1).broadcast_to([P, H, L])
        # pair-major ring: [pair q, half h, slot-in-pair w, k]; slot t>=1
        # lives at (q, w) = ((t-1)//2, (t-1)%2). Keeps every AP stride <= 64
        # (16-bit ISA stride fields) while dual-slot shuffle targets stay
        # flat contiguous 32-byte runs.
        ring4 = ring[:].rearrange("p (q h w k) -> p q h w k", h=2, w=2, k=H)
        iotf2 = iotf_sb[:].rearrange("p (h k) -> p h k", k=H)
        junk2 = junk[:].rearrange("p (h k) -> p h k", k=H)
        lens_bc = lens_sb[:, 0:1].broadcast_to([P, T_])
        HH = H // 2  # half of the own pages, for lt/reduce splitting
        N_SETUP = 7  # gpsimd setup ops counted on tbl_sem

        with nc.Block() as block:
            marks = {}
            total = [0]
            slice_marks = []

            @block.vector
            def _(v):
                n = [0]

                def S(inst):
                    inst.then_inc(dve_sem, 1)
                    n[0] += 1
                    return n[0]

                def W(k):
                    if k is not None:
                        v.wait_ge(dve_sem, k)

                v.wait_ge(tbl_sem, 16 * N_SETUP)
                # masks from lens: m[t] = t < len (used at t>=0),
                # nsm[t] = t >= len (only read at t>=1, where it equals the
                # reference's "masked step" indicator exactly)
                with nc.allow_low_precision(reason="0/1 mask in u8"):
                    S(v.tensor_tensor(out=m_sb[:], in0=it_sb[:], in1=lens_bc, op=A.is_lt))
                    S(v.tensor_tensor(out=nsm_sb[:], in0=it_sb[:], in1=lens_bc, op=A.is_ge))
                v.wait_ge(xa_sem, 16)  # chunk 0
                i_aown = S(v.tensor_copy(aown[:], xt_a[:, 0:H]))
                W(i_aown)
                S(v.stream_shuffle(afull[:, 0:H], aown[:], EVEN))
                i_shufO = S(v.stream_shuffle(afull[:, H:L], aown[:], ODD))

                # software-pipelined forward loop.
                # iteration t emits: alpha-chain(t), lt halves + reduce halves
                # of (t-1), ring shuffles of (t-2). Producer indices tracked
                # for exact wait targets.
                idx_pred = {}
                idx_shufO = {1: i_shufO}
                idx_scan = {}
                idx_ltb = {}
                idx_tt = {}
                idx_redb = {}

                def bpcol(s):
                    # bp_dual rotation: slot s lives at column (s+1)%4 so
                    # (odd, even) step pairs occupy contiguous column pairs.
                    return (s + 1) % 4

                for t in range(1, T_ + 3):
                    cur = t <= T_ - 1
                    c = t // CHUNK if cur else 0
                    u = t % CHUNK
                    xt = xt_ab[c % 2]
                    if cur and u == 0:
                        # first step of chunk c: ensure its DMA landed
                        # (issued ~CHUNK steps ago -> free wait)
                        if c % 2 == 0:
                            v.wait_ge(xa_sem, 16 * (c // 2 + 1))
                        else:
                            v.wait_ge(xb_sem, 16 * ((c - 1) // 2 + 1))

                    # --- alpha chain of t, software-pipelined with the
                    # backpointer side of steps t-1 / t-2 / t-3 / t-4 ---
                    if cur and t > 1:
                        W(idx_pred[t - 1])
                        S(v.stream_shuffle(afull[:, 0:H], aown[:], EVEN))
                        idx_shufO[t] = S(v.stream_shuffle(afull[:, H:L], aown[:], ODD))
                    # filler: lt first half of t-1 (reads R before scan(t))
                    s1 = t - 1
                    if 1 <= s1 <= T_ - 1:
                        W(idx_scan[s1])
                        S(v.tensor_tensor(
                            out=lt4[:, s1 % 2, 0:HH, :], in0=R3[:, 0:HH, :],
                            in1=Mv[:, 0:HH, :].broadcast_to([P, HH, L]), op=A.is_lt))
                    if cur:
                        W(idx_shufO[t])
                        i_add = S(v.tensor_tensor(out=sc3, in0=afull_b, in1=trep3, op=A.add))
                    # filler: lt second half of t-1
                    if 1 <= s1 <= T_ - 1:
                        idx_ltb[s1] = S(v.tensor_tensor(
                            out=lt4[:, s1 % 2, HH:H, :], in0=R3[:, HH:H, :],
                            in1=Mv[:, HH:H, :].broadcast_to([P, HH, L]), op=A.is_lt))
                    if cur:
                        W(i_add)
                        idx_scan[t] = S(v.tensor_tensor_scan(
                            out=R[:], data0=rstv_sb[:], data1=sc[:],
                            initial=0.0, op0=A.add, op1=A.max))
                    # filler: fp16 tree level of t-2
                    s2 = t - 2
                    if 1 <= s2 <= T_ - 1:
                        W(idx_ltb[s2])
                        idx_tt[s2] = S(v.tensor_tensor(
                            out=tt4[:, s2 % 2], in0=lt4[:, s2 % 2, :, 0:H],
                            in1=lt4[:, s2 % 2, :, H:L], op=A.add))
                    if cur:
                        W(idx_scan[t])
                        i_cand = S(v.tensor_tensor(
                            out=cand[:].unsqueeze(2), in0=Mv,
                            in1=xt[:, u * H : (u + 1) * H].unsqueeze(2), op=A.add))
                    # filler: reduce first half of t-3
                    s3 = t - 3
                    if 1 <= s3 <= T_ - 1:
                        W(idx_tt[s3])
                        with nc.allow_low_precision(reason="bp count <= 32, exact in u8"):
                            S(v.tensor_reduce(
                                out=bp4[:, bpcol(s3), 0:HH],
                                in_=tt4[:, s3 % 2, 0:HH, :], axis=AX.X, op=A.add))
                    if cur:
                        W(i_cand)
                        inst = v.copy_predicated(
                            out=aown[:],
                            mask=m_sb[:, t : t + 1].broadcast_to([P, H]),
                            data=cand[:])
                        idx_pred[t] = S(inst)
                        if u == CHUNK - 1:
                            # chunk c fully consumed by DVE at this point
                            marks[c] = n[0]
                    # tail filler: reduce second half of t-3
                    if 1 <= s3 <= T_ - 1:
                        with nc.allow_low_precision(reason="bp count <= 32, exact in u8"):
                            idx_redb[s3] = S(v.tensor_reduce(
                                out=bp4[:, bpcol(s3), HH:H],
                                in_=tt4[:, s3 % 2, HH:H, :], axis=AX.X, op=A.add))
                    # tail: dual-slot ring shuffles for the (s4-1, s4) pair
                    s4 = t - 4
                    if s4 >= 2 and s4 % 2 == 0 and s4 <= T_ - 1:
                        W(idx_redb[s4])
                        base = s4 % 4
                        bpp = bp_dual[:, base * H : (base + 2) * H]
                        q = s4 // 2 - 1
                        S(v.stream_shuffle(ring[:, q * 4 * H : q * 4 * H + 2 * H], bpp, EVEN))
                        S(v.stream_shuffle(ring[:, q * 4 * H + 2 * H : (q + 1) * 4 * H], bpp, ODD))

                # dangling odd final slot T-1 (2047): single-slot ring shuffles
                W(idx_redb[T_ - 1])
                bpl = bp4[:, bpcol(T_ - 1), :]
                ql = (T_ - 2) // 2
                S(v.stream_shuffle(ring4[:, ql, 0, 0, :], bpl, EVEN))
                S(v.stream_shuffle(ring4[:, ql, 1, 0, :], bpl, ODD))

                # --- identity backpointers on masked steps ---
                W(n[0])
                nq = T_ // 2
                for l_ in range(L):
                    for w in range(2):
                        qcnt = nq if w == 0 else nq - 1  # slot 2q+1+w <= T-1
                        S(v.copy_predicated(
                            out=ring4[:, 0:qcnt, l_ // H, w, l_ % H],
                            mask=nsm_sb[:, 1 + w : 2 + w + 2 * (qcnt - 1) : 2],
                            data=iotu_sb[:, l_ : l_ + 1].broadcast_to([P, qcnt])))

                # --- final argmax: paths[:, T-1] (first argmax of afull) ---
                # rebuild afull: the loop's last shuffles ran before pred(T-1),
                # so len==T sequences have a stale second... both halves.
                W(idx_pred[T_ - 1])
                S(v.stream_shuffle(afull[:, 0:H], aown[:], EVEN))
                S(v.stream_shuffle(afull[:, H:L], aown[:], ODD))
                W(n[0])
                S(v.tensor_tensor_scan(
                    out=lt32[:], data0=rstv_sb[:, 0:L], data1=afull[:],
                    initial=0.0, op0=A.add, op1=A.max))
                W(n[0])
                S(v.tensor_tensor(
                    out=junk[:], in0=lt32[:],
                    in1=lt32[:, L - 1 : L].broadcast_to([P, L]), op=A.is_lt))
                W(n[0])
                S(v.tensor_reduce(
                    out=paths[:, T_ - 1 : T_], in_=junk[:], axis=AX.X, op=A.add))

                # --- backtrace, with incremental masked-output slices ---
                # once the chase passes t, paths[t..] is final: fuse
                # mask*convert into one mixed TT (f32*u8 -> u8, exact for
                # tag ints) per slice and hand it to the DMA early so the
                # output transfer hides under the remaining chase.
                SLICE = max(64, T_ // 8)
                for t in range(T_ - 2, -1, -1):
                    W(n[0])
                    S(v.scalar_tensor_tensor(
                        out=junk2,
                        in0=iotf2,
                        scalar=paths[:, t + 1 : t + 2],
                        in1=ring4[:, t // 2, :, t % 2, :],
                        op0=A.is_equal,
                        op1=A.mult,
                        accum_out=paths[:, t : t + 1]))
                    if t % SLICE == 0:
                        W(n[0])
                        hi = T_ if t + SLICE > T_ - SLICE else t + SLICE
                        with nc.allow_low_precision(reason="tags 0..31 exact in u8"):
                            k = S(v.tensor_tensor(
                                out=outi[:, t:hi], in0=paths[:, t:hi],
                                in1=m_sb[:, t:hi], op=A.mult))
                        slice_marks.append((t, hi, k))
                total[0] = n[0]
                v.wait_ge(out_sem, 16 * len(slice_marks))

            @block.gpsimd
            def _(g):
                g.dma_start(trep_sb[:], trep[:]).then_inc(tbl_sem, 16)
                g.dma_start(lens_sb[:], lens[:]).then_inc(tbl_sem, 16)
                g.iota(it_sb[:], [[1, T_]], channel_multiplier=0,
                       allow_small_or_imprecise_dtypes=True).then_inc(tbl_sem, 16)
                g.iota(iotf_sb[:], [[1, L]], channel_multiplier=0,
                       allow_small_or_imprecise_dtypes=True).then_inc(tbl_sem, 16)
                with nc.allow_low_precision(reason="iota 0..31 exact in u8"):
                    g.tensor_copy(iotu_sb[:], iotf_sb[:]).then_inc(tbl_sem, 16)
                g.memset(rstv_sb[:], 0.0).then_inc(tbl_sem, 16)
                g.memset(rstv3[:, :, 0:1], NEG).then_inc(tbl_sem, 16)
                for c in range(nch):
                    if c >= 2:
                        g.wait_ge(dve_sem, marks[c - 2])
                    # chunk c carries only the pcs[c] leading partitions
                    # (sequences still alive at step 64c); stale rows beyond
                    # that are never selected by the masked alpha update
                    g.dma_start(
                        xt_ab[c % 2][0 : pcs[c], :],
                        x[row_off[c] : row_off[c + 1], :],
                    ).then_inc(xa_sem if c % 2 == 0 else xb_sem, 16)
                for (t0, hi, k) in slice_marks:
                    g.wait_ge(dve_sem, k)
                    g.dma_start(pout[:, t0:hi], outi[:, t0:hi]).then_inc(out_sem, 16)

    # the program is immutable from here on; memoize its BIR serialization
    # (the jit lowering re-serializes nc on every call otherwise, ~0.25s)
    blob = nc.to_json_bytes()
    nc.to_json_bytes = lambda: blob
    _NC_CACHE[key] = nc
    return nc


def prepare_in_maps(np_inputs, T_=T):
    inputs = np.asarray(np_inputs["inputs"], dtype=np.float32)
    seq_lengths = np.asarray(np_inputs["seq_lengths"], dtype=np.int32)
    trans_params = np.asarray(np_inputs["trans_params"], dtype=np.float32)

    tT = np.ascontiguousarray(trans_params.T)  # [j, i]
    trep = np.zeros((P, H, L), np.float32)
    trep[0::2] = tT[None, 0:H, :]
    trep[1::2] = tT[None, H:L, :]
    trep = trep.reshape(P, H * L)

    order, pcs = chunk_rows(seq_lengths, T_=T_)
    total_rows = int(sum(pcs))

    t_idx = np.arange(T_, dtype=np.int64)
    in_maps = []
    for k in range(NCORES):
        sel = order[np.arange(SEQ) * NCORES + k]      # pair j -> global seq
        xs = inputs[sel]                              # [SEQ, T, L]
        ls = seq_lengths[sel]                         # [SEQ], descending-ish
        # pair-split emissions: partition 2s = j 0..15, 2s+1 = j 16..31
        xo = np.empty((P, T_, H), np.float32)
        xo[0::2] = xs[:, :, 0:H]
        xo[1::2] = xs[:, :, H:L]
        # zero the padded region (never read by the decode; chunk-c rows a
        # core doesn't need are exactly these zeros, and they compress on
        # the wire)
        m = (t_idx[None, :] < ls[:, None])            # [SEQ, T]
        xo[np.repeat(~m, 2, axis=0)] = 0
        # pack: chunk c keeps only its first pcs[c] partitions
        xp = np.empty((total_rows, CE), np.float32)
        r = 0
        for c, pc in enumerate(pcs):
            xp[r : r + pc] = xo[0:pc, c * CHUNK : (c + 1) * CHUNK, :].reshape(pc, CE)
            r += pc
        lens = np.repeat(ls, 2).astype(np.float32).reshape(P, 1)
        in_maps.append({
            "x": xp,
            "trep": trep,
            "lens": lens,
        })
    meta = {"order": order, "pcs": pcs}
    return in_maps, meta


def assemble_output(results, meta):
    order = meta["order"]
    out = np.zeros((B, T), np.int32)
    for k in range(NCORES):
        sel = order[np.arange(SEQ) * NCORES + k]
        out[sel] = results[k]["paths"][0::2, :].astype(np.int32)
    return out


def kernel(inputs, seq_lengths, trans_params):
    in_maps, meta = prepare_in_maps(
        {
            "inputs": inputs,
            "seq_lengths": seq_lengths,
            "trans_params": trans_params,
        }
    )
    nc = build_program(pcs=meta["pcs"])
    res = run_bass_kernel_spmd(nc, in_maps, list(range(NCORES)))
    return assemble_output(res.results, meta)
